# revision 7
# baseline (speedup 1.0000x reference)
"""Trainium2 Bass kernel for nn_Attention3 (masked additive-MLP attention).

Math (per batch row b):
    h[p, :]    = relu(keys[b,p,:] @ W1k + query[b] @ W1q + frame[b] @ W1f)
    s[p]       = h[p, :] @ W2
    w          = softmax(where(mask==0, -inf, s))
    context    = w @ keys[b]
    out        = concat([context, frame[b]]), w

Sharding: batch B=256 data-parallel over 8 NeuronCores (32 rows each);
weights replicated. Host-side prep is layout-only (slicing + transposes of
the small query/frame shards and weight repacking).

Structure per core (32 rows, processed in 8 groups of 4):
  - keys group DMA'd as [128, 4, 512] (1 MB per transfer)
  - PE transposes keys chunks into PSUM (exact fp32), DVE/ACT copy to SBUF
  - hT[64, 512] = accumulated W1k^T @ keysT over 4 k-chunks (float32r,
    one matmul per chunk covering 4 batch rows at once)
  - ACT fuses +qf bias and ReLU per row; pair-packed score matmuls
    (rows stacked on partitions 0-63/64-127, W2 pair matrix) -> scoresT
  - one PE transpose -> scores [32, 128]; masked softmax (exp-zeroing)
  - context: 4 matmuls per group col-packed via tile_position into one
    PSUM tile (partitions 0/32/64/96), exact fp32
"""

import os
import sys
import numpy as np

for _p in ("/root/.axon_site", "/root/.axon_site/_ro/trn_rl_repo",
           "/root/.axon_site/_ro/pypackages", "/opt/trn_rl_repo"):
    if os.path.isdir(_p) and _p not in sys.path:
        sys.path.append(_p)

import concourse.bass as bass  # noqa: E402
import concourse.mybir as mybir  # noqa: E402
import concourse.tile as tile  # noqa: E402
from concourse import bacc  # noqa: E402
from concourse.bass_utils import run_bass_kernel_spmd  # noqa: E402

B, P, K, HE, HF = 256, 128, 512, 1024, 512
HID = 64
NCORES = 8
BC = B // NCORES  # 32 batch rows per core
NG = BC // 4      # 8 groups of 4 rows
F32 = mybir.dt.float32
F32R = mybir.dt.float32r
I32 = mybir.dt.int32
AF = mybir.ActivationFunctionType

HT_DT = F32R      # dtype of the big hT matmuls (float32r: 4x faster stream)
CTX_DT = F32      # dtype of the context matmuls (f32r forbids tile_position dst)
CTX_PACK = True   # col-pack context matmuls via tile_position

TRACE = False
LAST_EXEC_NS = None
_NC_CACHE = {}


def build_nc():
    nc = bacc.Bacc(None, target_bir_lowering=False)

    keys_d = nc.dram_tensor("keys", (BC, P, K), CTX_DT, kind="ExternalInput")
    qT_d = nc.dram_tensor("qT", (128, 8, BC), HT_DT, kind="ExternalInput")
    fT_d = nc.dram_tensor("fT", (128, 8, BC), HT_DT, kind="ExternalInput")
    fr_d = nc.dram_tensor("frame", (BC, 2 * HF), F32, kind="ExternalInput")
    mask_d = nc.dram_tensor("mask", (BC, P), I32, kind="ExternalInput")
    w1_d = nc.dram_tensor("W1t", (128, 20, HID), HT_DT, kind="ExternalInput")
    w2p_d = nc.dram_tensor("W2p", (128, 2), F32, kind="ExternalInput")
    id_d = nc.dram_tensor("ident", (128, 128), CTX_DT, kind="ExternalInput")
    idf_d = nc.dram_tensor("identf", (128, 128), F32, kind="ExternalInput")

    emb_d = nc.dram_tensor("emb", (BC, K + 2 * HF), F32, kind="ExternalOutput")
    wout_d = nc.dram_tensor("wout", (BC, P), F32, kind="ExternalOutput")

    with tile.TileContext(nc) as tc:
        with (
            tc.tile_pool(name="const", bufs=1) as const,
            tc.tile_pool(name="keys", bufs=NG) as keysp,
            tc.tile_pool(name="kT", bufs=2) as kTp,
            tc.tile_pool(name="hp", bufs=3) as hpp,
            tc.tile_pool(name="small", bufs=1) as small,
            tc.tile_pool(name="tp_ps", bufs=2, space="PSUM") as tp_ps,
            tc.tile_pool(name="hT_ps", bufs=2, space="PSUM") as hT_ps,
            tc.tile_pool(name="qf_ps", bufs=1, space="PSUM") as qf_psp,
            tc.tile_pool(name="scT_ps", bufs=1, space="PSUM") as scT_psp,
            tc.tile_pool(name="ctx_ps", bufs=2, space="PSUM") as ctx_ps,
        ):
            # ---- constants ----
            w1_sb = const.tile([128, 20, HID], HT_DT)
            nc.scalar.dma_start(w1_sb[:], w1_d[:])
            qT_sb = const.tile([128, 8, BC], HT_DT)
            nc.scalar.dma_start(qT_sb[:], qT_d[:])
            fT_sb = const.tile([128, 8, BC], HT_DT)
            nc.scalar.dma_start(fT_sb[:], fT_d[:])
            w2p_sb = const.tile([128, 2], F32)
            nc.scalar.dma_start(w2p_sb[:], w2p_d[:])
            id_sb = const.tile([128, 128], CTX_DT)
            nc.scalar.dma_start(id_sb[:], id_d[:])
            idf_sb = const.tile([128, 128], F32)
            nc.scalar.dma_start(idf_sb[:], idf_d[:])
            mask_sb = const.tile([BC, P], I32)
            nc.scalar.dma_start(mask_sb[:], mask_d[:])

            # frame passthrough early on the SWDGE path (no deps)
            nc.gpsimd.dma_start(emb_d[:, K:], fr_d[:])

            # ---- keys: fully resident, 4 batch rows (1 MB) per DMA ----
            keys_sb = []
            for g in range(NG):
                t = keysp.tile([128, 4, K], CTX_DT, tag="keys")
                nc.sync.dma_start(
                    t[:], keys_d[4 * g : 4 * g + 4].rearrange("b p k -> p b k")
                )
                keys_sb.append(t)

            def keys_row(b):
                return keys_sb[b // 4][:, b % 4, :]

            # ---- qf[j, b] = (query @ W1q + frame @ W1f)^T : [64, BC] ----
            qf_ps = qf_psp.tile([HID, BC], F32, tag="qf")
            for o in range(8):
                nc.tensor.matmul(
                    qf_ps[:], w1_sb[:, 4 + o, :], qT_sb[:, o, :],
                    start=(o == 0), stop=False,
                )
            for o in range(8):
                nc.tensor.matmul(
                    qf_ps[:], w1_sb[:, 12 + o, :], fT_sb[:, o, :],
                    start=False, stop=(o == 7),
                )
            qf_sb = small.tile([HID, BC], F32)
            nc.vector.tensor_copy(qf_sb[:], qf_ps[:])

            # ---- per group: keysT -> hT -> relu -> pair scores ----
            scT_ps = scT_psp.tile([128, BC], F32, tag="scT")
            for g in range(NG):
                kg = keys_sb[g]
                kT_sb = kTp.tile([128, 4, K], HT_DT, tag="kT")
                for c in range(4):
                    tp = tp_ps.tile([128, K], CTX_DT, tag="tp")
                    for b in range(4):
                        nc.tensor.transpose(
                            tp[:, 128 * b : 128 * (b + 1)],
                            kg[:, b, 128 * c : 128 * (c + 1)],
                            id_sb[:],
                        )
                    eng = nc.vector if c % 2 == 0 else nc.scalar
                    if c % 2 == 0:
                        nc.vector.tensor_copy(kT_sb[:, c, :], tp[:])
                    else:
                        nc.scalar.activation(kT_sb[:, c, :], tp[:], AF.Copy)

                hT = hT_ps.tile([HID, 4 * P], F32, tag="hT")
                for c in range(4):
                    nc.tensor.matmul(
                        hT[:], w1_sb[:, c, :], kT_sb[:, c, :],
                        start=(c == 0), stop=(c == 3),
                    )

                for p2 in range(2):  # pairs (4g+0,4g+1), (4g+2,4g+3)
                    hp_sb = hpp.tile([128, P], F32, tag="hp")
                    for i in range(2):
                        b = 4 * g + 2 * p2 + i
                        nc.scalar.activation(
                            hp_sb[64 * i : 64 * i + 64, :],
                            hT[:, 128 * (2 * p2 + i) : 128 * (2 * p2 + i + 1)],
                            AF.Relu,
                            bias=qf_sb[:, b : b + 1],
                        )
                    col = 4 * g + 2 * p2
                    nc.tensor.matmul(
                        scT_ps[:, col : col + 2], hp_sb[:], w2p_sb[:],
                        start=True, stop=True,
                    )

            # ---- scores [BC, P]: transpose scT then masked softmax ----
            scT_sb = small.tile([128, BC], F32)
            nc.vector.tensor_copy(scT_sb[:], scT_ps[:])
            sc_ps = hT_ps.tile([BC, P], F32, tag="hT")
            nc.tensor.transpose(sc_ps[:], scT_sb[:], idf_sb[:])
            s_sb = small.tile([BC, P], F32)
            nc.vector.tensor_copy(s_sb[:], sc_ps[:])

            mf_sb = small.tile([BC, P], F32)
            nc.vector.tensor_copy(mf_sb[:], mask_sb[:])  # int32 -> fp32
            gmax = small.tile([BC, 1], F32)
            nc.vector.reduce_max(gmax[:], s_sb[:], axis=mybir.AxisListType.X)
            nc.vector.tensor_scalar_mul(gmax[:], gmax[:], -1.0)
            e_sb = small.tile([BC, P], F32)
            nc.scalar.activation(e_sb[:], s_sb[:], AF.Exp, bias=gmax[:, 0:1])
            nc.vector.tensor_mul(out=e_sb[:], in0=e_sb[:], in1=mf_sb[:])
            ssum = small.tile([BC, 1], F32)
            nc.vector.reduce_sum(ssum[:], e_sb[:], axis=mybir.AxisListType.X)
            rinv = small.tile([BC, 1], F32)
            nc.vector.reciprocal(rinv[:], ssum[:])
            w_sb = small.tile([BC, P], F32)
            nc.vector.tensor_scalar_mul(w_sb[:], e_sb[:], rinv[:, 0:1])
            nc.scalar.dma_start(wout_d[:], w_sb[:])

            # ---- context: wT columns stationary, stream keys rows ----
            wT_ps = tp_ps.tile([P, BC], CTX_DT, tag="tp")
            wTin_sb = small.tile([BC, P], CTX_DT)
            nc.vector.tensor_copy(wTin_sb[:], w_sb[:])
            nc.tensor.transpose(wT_ps[:], wTin_sb[:], id_sb[:BC, :BC])
            wT_sb = small.tile([P, BC], CTX_DT)
            nc.vector.tensor_copy(wT_sb[:], wT_ps[:])

            if CTX_PACK:
                ctx_sb = small.tile([128, NG, K], F32)
                for g in range(NG):
                    cp = ctx_ps.tile([128, K], F32, tag="ctx")
                    for j in range(4):
                        b = 4 * g + j
                        nc.tensor.matmul(
                            cp[32 * j : 32 * j + 1, :],
                            wT_sb[:, b : b + 1],
                            keys_row(b),
                            start=True, stop=True,
                            tile_position=(0, 32 * j),
                        )
                    nc.vector.tensor_copy(ctx_sb[:, g, :], cp[:])
                nc.scalar.dma_start(
                    emb_d[:, 0:K].rearrange("(g j) k -> j g k", j=4),
                    ctx_sb.rearrange("(j r) g k -> j r g k", r=32)[:, 0, :, :],
                )
            else:
                ctx_sb = small.tile([1, BC * K], F32)
                for b in range(BC):
                    cp = ctx_ps.tile([1, K], F32, tag="ctx")
                    nc.tensor.matmul(
                        cp[:], wT_sb[:, b : b + 1], keys_row(b),
                        start=True, stop=True,
                    )
                    nc.scalar.activation(
                        ctx_sb[0:1, K * b : K * (b + 1)], cp[:], AF.Copy
                    )
                nc.sync.dma_start(
                    emb_d[:, 0:K][None],
                    ctx_sb.rearrange("p (b k) -> p b k", k=K),
                )


    nc.compile()
    return nc


def _get_nc():
    if "nc" not in _NC_CACHE:
        _NC_CACHE["nc"] = build_nc()
    return _NC_CACHE["nc"]


def kernel(query, keys, frameLSTM_h, mask, W1, W2):
    global LAST_EXEC_NS
    query = np.asarray(query, dtype=np.float32).reshape(B, HE)
    keys = np.asarray(keys, dtype=np.float32)
    frame = np.asarray(frameLSTM_h, dtype=np.float32).reshape(B, 2 * HF)
    mask = np.asarray(mask, dtype=np.int32)
    W1 = np.asarray(W1, dtype=np.float32)
    W2 = np.asarray(W2, dtype=np.float32)

    # replicated weights, repacked for 128-partition tiles (layout only)
    W1t = np.ascontiguousarray(W1.reshape(20, 128, HID).transpose(1, 0, 2))
    W2p = np.zeros((128, 2), dtype=np.float32)
    W2p[:HID, 0] = W2[:, 0]
    W2p[HID:, 1] = W2[:, 0]
    ident = np.eye(128, dtype=np.float32)

    def tshard(x, c):  # (BC, D) -> (128, D//128, BC) partition-chunked transpose
        xT = np.ascontiguousarray(x[c * BC : (c + 1) * BC].T)  # (D, BC)
        return np.ascontiguousarray(
            xT.reshape(-1, 128, BC).transpose(1, 0, 2)
        )

    in_maps = []
    for c in range(NCORES):
        sl = slice(c * BC, (c + 1) * BC)
        in_maps.append(
            {
                "keys": np.ascontiguousarray(keys[sl]),
                "qT": tshard(query, c),
                "fT": tshard(frame, c),
                "frame": np.ascontiguousarray(frame[sl]),
                "mask": np.ascontiguousarray(mask[sl]),
                "W1t": W1t,
                "W2p": W2p,
                "ident": ident,
                "identf": ident,
            }
        )

    nc = _get_nc()
    kwargs = {}
    if TRACE:
        kwargs = dict(trace=True)
    res = run_bass_kernel_spmd(nc, in_maps, core_ids=list(range(NCORES)), **kwargs)
    LAST_EXEC_NS = res.exec_time_ns

    embeddings = np.concatenate([r["emb"] for r in res.results], axis=0)
    weights = np.concatenate([r["wout"] for r in res.results], axis=0)
    return embeddings.reshape(B, 1, K + 2 * HF), weights


# revision 10
# speedup vs baseline: 1.0502x; 1.0502x over previous
"""Trainium2 Bass kernel for nn_Attention3 (masked additive-MLP attention).

Math (per batch row b):
    h[p, :]    = relu(keys[b,p,:] @ W1k + query[b] @ W1q + frame[b] @ W1f)
    s[p]       = h[p, :] @ W2
    w          = softmax(where(mask==0, -inf, s))
    context    = w @ keys[b]
    out        = concat([context, frame[b]]), w

Sharding: batch B=256 data-parallel over 8 NeuronCores (32 rows each);
weights replicated. Host-side prep is layout-only (slicing, transposes of
the small query/frame shards, weight repacking, mask int->float).

Structure per core (32 rows, 8 groups of 4):
  - constants packed into two wide DMAs (one f32r, one f32)
  - keys groups [128, 4, 512] (1 MB each), alternating HWDGE queues
  - PE transposes keys chunks into PSUM (exact fp32), DVE/ACT copy out
  - hT[64, 512] accumulated over k-chunks in float32r (4x stream rate)
  - ACT fuses +qf bias + ReLU; pair-packed score matmuls -> scoresT
  - one PE transpose -> scores [32, 128]; masked softmax (exp-zeroing,
    fused mask-mult + row-sum)
  - context: exact fp32, 4 matmuls per group col-packed via tile_position
    (PSUM partitions 0/32/64/96), per-group output DMA
"""

import os
import sys
import numpy as np

for _p in ("/root/.axon_site", "/root/.axon_site/_ro/trn_rl_repo",
           "/root/.axon_site/_ro/pypackages", "/opt/trn_rl_repo"):
    if os.path.isdir(_p) and _p not in sys.path:
        sys.path.append(_p)

import concourse.bass as bass  # noqa: E402
import concourse.mybir as mybir  # noqa: E402
import concourse.tile as tile  # noqa: E402
from concourse import bacc  # noqa: E402
from concourse.bass_utils import run_bass_kernel_spmd  # noqa: E402

B, P, K, HE, HF = 256, 128, 512, 1024, 512
HID = 64
NCORES = 8
BC = B // NCORES  # 32 batch rows per core
NG = BC // 4      # 8 groups of 4 rows
F32 = mybir.dt.float32
F32R = mybir.dt.float32r
AF = mybir.ActivationFunctionType
ALU = mybir.AluOpType

TRACE = False
LAST_EXEC_NS = None
_NC_CACHE = {}


def build_nc():
    nc = bacc.Bacc(None, target_bir_lowering=False)

    keys_d = nc.dram_tensor("keys", (BC, P, K), F32, kind="ExternalInput")
    cr_d = nc.dram_tensor("cr", (128, 1792), F32R, kind="ExternalInput")
    cf_d = nc.dram_tensor("cf", (128, 130), F32, kind="ExternalInput")
    mk_d = nc.dram_tensor("maskf", (BC, P), F32, kind="ExternalInput")
    fr_d = nc.dram_tensor("frame", (BC, 2 * HF), F32, kind="ExternalInput")

    emb_d = nc.dram_tensor("emb", (BC, K + 2 * HF), F32, kind="ExternalOutput")
    wout_d = nc.dram_tensor("wout", (BC, P), F32, kind="ExternalOutput")

    with tile.TileContext(nc) as tc:
        with (
            tc.tile_pool(name="const", bufs=1) as const,
            tc.tile_pool(name="keys", bufs=NG) as keysp,
            tc.tile_pool(name="kT", bufs=2) as kTp,
            tc.tile_pool(name="hp", bufs=3) as hpp,
            tc.tile_pool(name="small", bufs=1) as small,
            tc.tile_pool(name="tp_ps", bufs=2, space="PSUM") as tp_ps,
            tc.tile_pool(name="hT_ps", bufs=2, space="PSUM") as hT_ps,
            tc.tile_pool(name="qf_ps", bufs=1, space="PSUM") as qf_psp,
            tc.tile_pool(name="scT_ps", bufs=1, space="PSUM") as scT_psp,
            tc.tile_pool(name="ctx_ps", bufs=2, space="PSUM") as ctx_ps,
        ):
            # ---- constants: packed DMAs on the scalar queue ----
            cr_sb = const.tile([128, 1792], F32R)
            nc.scalar.dma_start(cr_sb[:], cr_d[:])
            cf_sb = const.tile([128, 130], F32)
            nc.scalar.dma_start(cf_sb[:], cf_d[:])
            mf_sb = const.tile([BC, P], F32)
            nc.scalar.dma_start(mf_sb[:], mk_d[:])

            w1_sb = cr_sb[:, 0:1280].rearrange("p (o h) -> p o h", h=HID)
            qT_sb = cr_sb[:, 1280:1536].rearrange("p (o b) -> p o b", b=BC)
            fT_sb = cr_sb[:, 1536:1792].rearrange("p (o b) -> p o b", b=BC)
            id_sb = cf_sb[:, 0:128]
            w2p_sb = cf_sb[:, 128:130]

            # frame passthrough early on the SWDGE path (no deps)
            nc.gpsimd.dma_start(emb_d[:, K:], fr_d[:])

            # ---- keys: fully resident, 1 MB per DMA, alternate queues ----
            keys_sb = []
            for g in range(NG):
                t = keysp.tile([128, 4, K], F32, tag="keys")
                eng = nc.sync if g % 2 == 0 else nc.scalar
                eng.dma_start(
                    t[:], keys_d[4 * g : 4 * g + 4].rearrange("b p k -> p b k")
                )
                keys_sb.append(t)

            def keys_row(b):
                return keys_sb[b // 4][:, b % 4, :]

            # ---- qf[j, b] = (query @ W1q + frame @ W1f)^T : [64, BC] ----
            qf_ps = qf_psp.tile([HID, BC], F32, tag="qf")
            for o in range(8):
                nc.tensor.matmul(
                    qf_ps[:], w1_sb[:, 4 + o, :], qT_sb[:, o, :],
                    start=(o == 0), stop=False,
                )
            for o in range(8):
                nc.tensor.matmul(
                    qf_ps[:], w1_sb[:, 12 + o, :], fT_sb[:, o, :],
                    start=False, stop=(o == 7),
                )
            qf_sb = small.tile([HID, BC], F32)
            nc.vector.tensor_copy(qf_sb[:], qf_ps[:])

            # ---- per group: keysT -> hT -> relu -> pair scores ----
            scT_ps = scT_psp.tile([128, BC], F32, tag="scT")
            for g in range(NG):
                kg = keys_sb[g]
                kT_sb = kTp.tile([128, 4, K], F32R, tag="kT")
                for c in range(4):
                    tp = tp_ps.tile([128, K], F32, tag="tp")
                    for b in range(4):
                        nc.tensor.transpose(
                            tp[:, 128 * b : 128 * (b + 1)],
                            kg[:, b, 128 * c : 128 * (c + 1)],
                            id_sb[:],
                        )
                    if c % 2 == 0:
                        nc.vector.tensor_copy(kT_sb[:, c, :], tp[:])
                    else:
                        nc.scalar.activation(kT_sb[:, c, :], tp[:], AF.Copy)

                hT = hT_ps.tile([HID, 4 * P], F32, tag="hT")
                for c in range(4):
                    nc.tensor.matmul(
                        hT[:], w1_sb[:, c, :], kT_sb[:, c, :],
                        start=(c == 0), stop=(c == 3),
                    )

                for p2 in range(2):  # pairs (4g+0,4g+1), (4g+2,4g+3)
                    hp_sb = hpp.tile([128, P], F32, tag="hp")
                    for i in range(2):
                        b = 4 * g + 2 * p2 + i
                        nc.scalar.activation(
                            hp_sb[64 * i : 64 * i + 64, :],
                            hT[:, 128 * (2 * p2 + i) : 128 * (2 * p2 + i + 1)],
                            AF.Relu,
                            bias=qf_sb[:, b : b + 1],
                        )
                    col = 4 * g + 2 * p2
                    nc.tensor.matmul(
                        scT_ps[:, col : col + 2], hp_sb[:], w2p_sb[:],
                        start=True, stop=True,
                    )

            # ---- scores [BC, P]: transpose scT then masked softmax ----
            scT_sb = small.tile([128, BC], F32)
            nc.vector.tensor_copy(scT_sb[:], scT_ps[:])
            sc_ps = hT_ps.tile([BC, P], F32, tag="hT")
            nc.tensor.transpose(sc_ps[:], scT_sb[:], id_sb[:])

            gmax = small.tile([BC, 1], F32)
            nc.vector.reduce_max(gmax[:], sc_ps[:], axis=mybir.AxisListType.X)
            nc.vector.tensor_scalar_mul(gmax[:], gmax[:], -1.0)
            e_sb = small.tile([BC, P], F32)
            nc.scalar.activation(e_sb[:], sc_ps[:], AF.Exp, bias=gmax[:, 0:1])
            ssum = small.tile([BC, 1], F32)
            nc.vector.tensor_mul(out=e_sb[:], in0=e_sb[:], in1=mf_sb[:])
            nc.vector.reduce_sum(ssum[:], e_sb[:], axis=mybir.AxisListType.X)
            rinv = small.tile([BC, 1], F32)
            nc.vector.reciprocal(rinv[:], ssum[:])
            w_sb = small.tile([BC, P], F32)
            nc.vector.tensor_scalar_mul(w_sb[:], e_sb[:], rinv[:, 0:1])
            nc.scalar.dma_start(wout_d[:], w_sb[:])

            # ---- context: wT columns stationary, stream keys rows ----
            wT_ps = tp_ps.tile([P, BC], F32, tag="tp")
            nc.tensor.transpose(wT_ps[:], w_sb[:], id_sb[:BC, :BC])
            wT_sb = small.tile([P, BC], F32)
            nc.vector.tensor_copy(wT_sb[:], wT_ps[:])

            ctx_sb = small.tile([128, NG, K], F32)
            for g in range(NG):
                cp = ctx_ps.tile([128, K], F32, tag="ctx")
                for j in range(4):
                    b = 4 * g + j
                    nc.tensor.matmul(
                        cp[32 * j : 32 * j + 1, :],
                        wT_sb[:, b : b + 1],
                        keys_row(b),
                        start=True, stop=True,
                        tile_position=(0, 32 * j),
                    )
                nc.vector.tensor_copy(ctx_sb[:, g, :], cp[:])
                nc.scalar.dma_start(
                    emb_d[4 * g : 4 * g + 4, 0:K].rearrange("j (u k) -> j u k", u=1),
                    ctx_sb.rearrange("(j r) g k -> j r g k", r=32)[
                        :, 0, g : g + 1, :
                    ],
                )

    nc.compile()
    return nc


def _get_nc():
    if "nc" not in _NC_CACHE:
        _NC_CACHE["nc"] = build_nc()
    return _NC_CACHE["nc"]


def kernel(query, keys, frameLSTM_h, mask, W1, W2):
    global LAST_EXEC_NS
    query = np.asarray(query, dtype=np.float32).reshape(B, HE)
    keys = np.asarray(keys, dtype=np.float32)
    frame = np.asarray(frameLSTM_h, dtype=np.float32).reshape(B, 2 * HF)
    mask = np.asarray(mask)
    W1 = np.asarray(W1, dtype=np.float32)
    W2 = np.asarray(W2, dtype=np.float32)

    # replicated weights, repacked for 128-partition tiles (layout only)
    W1t = np.ascontiguousarray(
        W1.reshape(20, 128, HID).transpose(1, 0, 2)
    ).reshape(128, 1280)
    W2p = np.zeros((128, 2), dtype=np.float32)
    W2p[:HID, 0] = W2[:, 0]
    W2p[HID:, 1] = W2[:, 0]
    ident = np.eye(128, dtype=np.float32)
    cf = np.concatenate([ident, W2p], axis=1)  # (128, 130)

    def tshard(x, c):  # (BC, D) -> (128, D//128 * BC) partition-chunked transpose
        xT = np.ascontiguousarray(x[c * BC : (c + 1) * BC].T)  # (D, BC)
        return np.ascontiguousarray(
            xT.reshape(-1, 128, BC).transpose(1, 0, 2)
        ).reshape(128, -1)

    maskf = mask.astype(np.float32)

    in_maps = []
    for c in range(NCORES):
        sl = slice(c * BC, (c + 1) * BC)
        cr = np.concatenate([W1t, tshard(query, c), tshard(frame, c)], axis=1)
        in_maps.append(
            {
                "keys": np.ascontiguousarray(keys[sl]),
                "cr": np.ascontiguousarray(cr),
                "cf": cf,
                "maskf": np.ascontiguousarray(maskf[sl]),
                "frame": np.ascontiguousarray(frame[sl]),
            }
        )

    nc = _get_nc()
    kwargs = {}
    if TRACE:
        kwargs = dict(trace=True)
    res = run_bass_kernel_spmd(nc, in_maps, core_ids=list(range(NCORES)), **kwargs)
    LAST_EXEC_NS = res.exec_time_ns

    embeddings = np.concatenate([r["emb"] for r in res.results], axis=0)
    weights = np.concatenate([r["wout"] for r in res.results], axis=0)
    return embeddings.reshape(B, 1, K + 2 * HF), weights
